# revision 41
# baseline (speedup 1.0000x reference)
"""Trainium2 Bass kernel: multi-head self-attention block (B=16, N=1024, C=768, H=12).

Data-parallel over batch: 8 NeuronCores x 2 batches each, no collectives.

Restructured schedule v2 (vs v1 which ran phases back to back):
  * 12 "units" = (batch, head-pair), batch-major. Attention for unit i is
    exp-paced on the Scalar engine; all other PE work (qk projection for
    unit i+1, v-projection, output projection of batch 0) is interleaved
    as "filler" groups so the PE never idle-waits on exp/psum deps.
  * The first exp fires ~13us in (qk(unit0) streams as soon as its DMA
    lands) instead of ~59us.
  * U matmuls for unit i run as one tight burst at the start of unit i+1,
    when all 16 E tiles are already in SBUF -> no exp-dependency stalls.
  * PSUM: 2x[128,1024] S slots (4 banks) + a 4x1-bank "flex" ring shared
    by U accumulators, qk/v/proj groups, time-sliced within each unit.

Dataflow per core (all-transposed activations; no on-chip transposes):
  host: xT = x_shard^T                                  [C, T]
  qT/kT(hp,b) = Wq/Wk^T-slices @ xT(b)                  [128, N]
  v'   = xT-tiles^T @ W_qkv[:, 2C:]  (+ ones col/head)  [N, H*(HD+1)]
  S^T  = k^T-slices^T @ q^T   (per head, K=64)          [Nk, Nq]
  E    = exp(SCALE * S^T)     (ScalarE, PSUM->SBUF)
  U'   = v'^T @ E  (accum over k; row HD = softmax Z)   [HD+1, Nq]
  aoT  = U'[:HD] * (1/Z broadcast)                      [C, N]
  y    = aoT-tiles^T @ W_proj + b                       [N, C]
"""

import sys

for _p in ("/opt/trn_rl_repo", "/opt/pypackages"):
    if _p not in sys.path:
        sys.path.append(_p)

import numpy as np

B, N, C, H = 16, 1024, 768, 12
HD = C // H            # 64
SCALE = HD ** -0.5
NCORES = 8
BL = B // NCORES       # 2 batches per core
T = BL * N             # 2048 tokens per core

COMPUTE = "bf16"       # "bf16" | "f32" | "f32r"


def build_attention_nc(compute=COMPUTE, bl=BL, n=N, c=C, h=H):
    import concourse.bass as bass
    import concourse.tile as tile
    from concourse import bacc, mybir
    from contextlib import ExitStack

    hd = c // h
    t = bl * n
    scale = hd ** -0.5
    assert c % 128 == 0 and n % 512 == 0 and h % 2 == 0 and hd == 64
    CCH = c // 128      # contraction chunks over channels (6)
    NHP = h // 2        # head pairs (6)
    NQ = n // 512       # 512-wide q tiles per sequence (2)
    NKT = n // 128      # 128-wide k tiles per sequence (8)
    NTT = n // 128      # 128-wide token tiles per sequence (8)
    VW = hd + 1         # v' width per head (ones col at hd)
    PH = c // 2         # proj/v free-dim half (384), <= 1 PSUM bank
    NXH = n // 512      # 512-col x halves per batch (2)

    FP32 = mybir.dt.float32
    SD = mybir.dt.bfloat16 if compute == "bf16" else FP32  # storage dtype

    def mm(ap):
        return ap.bitcast(mybir.dt.float32r) if compute == "f32r" else ap

    nc = bacc.Bacc("TRN2", target_bir_lowering=False, debug=False,
                   num_devices=NCORES)

    xT_d = nc.dram_tensor("xT", [c, t], SD, kind="ExternalInput").ap()
    wqkv_d = nc.dram_tensor("w_qkv", [c, 3 * c], SD, kind="ExternalInput").ap()
    wproj_d = nc.dram_tensor("w_proj", [c, c], SD, kind="ExternalInput").ap()
    bias_d = nc.dram_tensor("bias", [128, c], FP32, kind="ExternalInput").ap()
    out_d = nc.dram_tensor("out", [t, c], FP32, kind="ExternalOutput").ap()

    Exp = mybir.ActivationFunctionType.Exp

    units = [(b, hp) for b in range(bl) for hp in range(NHP)]
    NU = len(units)     # 12

    with tile.TileContext(nc) as tc, ExitStack() as ctx:
        consts = ctx.enter_context(tc.tile_pool(name="consts", bufs=1))
        xp = ctx.enter_context(tc.tile_pool(name="xp", bufs=2))
        qkp = ctx.enter_context(tc.tile_pool(name="qkp", bufs=3))
        vp = ctx.enter_context(tc.tile_pool(name="vp", bufs=2))
        ep = ctx.enter_context(tc.tile_pool(name="ep", bufs=8))
        aop = ctx.enter_context(tc.tile_pool(name="aop", bufs=2))
        smp = ctx.enter_context(tc.tile_pool(name="smp", bufs=1))
        yp = ctx.enter_context(tc.tile_pool(name="yp", bufs=4))
        ps_s = ctx.enter_context(tc.tile_pool(name="ps_s", bufs=2, space="PSUM"))
        ps_f = ctx.enter_context(tc.tile_pool(name="ps_f", bufs=4, space="PSUM"))

        # ---------------- DMA prologue --------------------------------------
        # Flat contiguous per-chunk descriptors (strided "slab" descriptors
        # transfer at ~1/3 the bandwidth). Ramp-critical stream on the sync
        # queue ordered so the first qk groups chase it chunk by chunk;
        # bulk on the gpsimd queue.
        wq_hp0 = [consts.tile([128, 128], SD, tag=f"wqh{cc}",
                              name=f"wqh0_{cc}") for cc in range(CCH)]
        wk_hp0 = [consts.tile([128, 128], SD, tag=f"wkh{cc}",
                              name=f"wkh0_{cc}") for cc in range(CCH)]
        xtile = [[None] * CCH for _ in range(bl)]
        xT_all = [[[None] * NXH for _ in range(CCH)] for _ in range(bl)]
        for b in range(bl):
            for cc in range(CCH):
                xt = xp.tile([128, n], SD, tag=f"x{cc}", name=f"x_b{b}c{cc}")
                xtile[b][cc] = xt
                for xh in range(NXH):
                    xT_all[b][cc][xh] = xt[:, xh * 512:(xh + 1) * 512]
        for cc in range(CCH):
            r0, r1 = cc * 128, (cc + 1) * 128
            q = nc.sync if cc < 3 else nc.gpsimd
            q.dma_start(out=wq_hp0[cc], in_=wqkv_d[r0:r1, 0:128])
            q.dma_start(out=wk_hp0[cc], in_=wqkv_d[r0:r1, c:c + 128])
            q.dma_start(out=xtile[0][cc], in_=xT_d[r0:r1, 0:n])
        # wv (v(b0) follows as unit-0 filler)
        wv_sb = [consts.tile([128, c], SD, tag=f"wv{cc}", name=f"wv{cc}")
                 for cc in range(CCH)]
        for cc in range(CCH):
            nc.sync.dma_start(out=wv_sb[cc],
                              in_=wqkv_d[cc * 128:(cc + 1) * 128, 2 * c:3 * c])
        # xT batch 1 + remaining weights go on the gpsimd queue, but only
        # after the ramp-critical sync stream has landed (the two queues
        # share HBM bandwidth) — see the gate op after the prologue qk.
        wq_sb = [consts.tile([128, c], SD, tag=f"wq{cc}", name=f"wq{cc}")
                 for cc in range(CCH)]
        wk_sb = [consts.tile([128, c], SD, tag=f"wk{cc}", name=f"wk{cc}")
                 for cc in range(CCH)]
        wproj_sb = [consts.tile([128, c], SD, tag=f"wp{cc}", name=f"wp{cc}")
                    for cc in range(CCH)]
        for cc in range(CCH):
            nc.sync.dma_start(out=xtile[1][cc],
                              in_=xT_d[cc * 128:(cc + 1) * 128, n:2 * n])
        # gate: the gpsimd weight bulk waits for the last ramp-critical
        # x(b0) tile, so it doesn't steal HBM bandwidth from the ramp
        gate = smp.tile([1, 16], SD, tag="gate")
        nc.gpsimd.tensor_copy(gate, xtile[0][2][0:1, 0:16])
        for cc in range(CCH):
            r0, r1 = cc * 128, (cc + 1) * 128
            nc.gpsimd.dma_start(out=wq_sb[cc], in_=wqkv_d[r0:r1, 0:c])
            nc.gpsimd.dma_start(out=wk_sb[cc], in_=wqkv_d[r0:r1, c:2 * c])
            nc.gpsimd.dma_start(out=wproj_sb[cc], in_=wproj_d[r0:r1, :])
        bias_sb = consts.tile([128, c], FP32, tag="bias")
        nc.gpsimd.dma_start(out=bias_sb, in_=bias_d)

        # ---------------- building-block emitters --------------------------
        qt_all = {}   # (b, hp) -> [128, n] q^T tile (2 heads stacked)
        kt_all = {}
        v_all = [[None] * NTT for _ in range(bl)]
        e_all = {}    # (b, hp, kt, head) -> E tile
        u_ps = {}     # (b, hp) -> [head][qn] psum accumulators
        ao_all = {}   # (b, hp) -> [128, n] normalized attention output^T

        def emit_qk_group(b, hp, dst, qn):
            """Project one 512-token slice of q^T (dst=0) or k^T (dst=1)."""
            key = (b, hp)
            store = qt_all if dst == 0 else kt_all
            if key not in store:
                store[key] = qkp.tile([128, n], SD, tag=f"qk{dst}",
                                      name=f"{'qk'[dst]}t_b{b}hp{hp}")
            ps = ps_f.tile([128, 512], FP32, tag="u",
                           name=f"qkps_b{b}hp{hp}d{dst}q{qn}")
            for cc in range(CCH):
                if b == 0 and hp == 0:
                    w_ap = (wq_hp0 if dst == 0 else wk_hp0)[cc]
                else:
                    w_sb = wq_sb if dst == 0 else wk_sb
                    w_ap = w_sb[cc][:, hp * 128:(hp + 1) * 128]
                nc.tensor.matmul(
                    ps,
                    lhsT=mm(w_ap),
                    rhs=mm(xT_all[b][cc][qn]),
                    start=(cc == 0), stop=(cc == CCH - 1))
            with tc.high_priority(offset=300):
                nc.vector.tensor_copy(
                    store[key][:, qn * 512:(qn + 1) * 512], ps)

        def emit_v_group(b, tt, half):
            """One [128-token, 384-channel] slice of v' (+ones cols)."""
            if half == 0:
                vt = vp.tile([128, h * VW], SD, tag=f"v{tt}",
                             name=f"v_b{b}t{tt}")
                ones_view = vt[:, :].rearrange(
                    "p (hh w) -> p hh w", hh=h)[:, :, hd:hd + 1]
                nc.gpsimd.memset(ones_view, 1.0)
                v_all[b][tt] = vt
            vt = v_all[b][tt]
            ps = ps_f.tile([128, PH], FP32, tag="u",
                           name=f"vps_b{b}t{tt}f{half}")
            xh, tl = tt // 4, tt % 4
            for cc in range(CCH):
                nc.tensor.matmul(
                    ps,
                    lhsT=mm(xT_all[b][cc][xh][:, tl * 128:(tl + 1) * 128]),
                    rhs=mm(wv_sb[cc][:, half * PH:(half + 1) * PH]),
                    start=(cc == 0), stop=(cc == CCH - 1))
            nheads = PH // hd
            dst = vt[:, half * nheads * VW:(half + 1) * nheads * VW].rearrange(
                "p (hh w) -> p hh w", hh=nheads)[:, :, 0:hd]
            srcv = ps[:].rearrange("p (hh w) -> p hh w", hh=nheads)
            with tc.high_priority(offset=300):
                nc.vector.tensor_copy(dst, srcv)

        def emit_proj_group(b, tt, half):
            """One [128-token, 384-channel] output-projection slice (+bias)."""
            ps = ps_f.tile([128, PH], FP32, tag="u",
                           name=f"yps_b{b}t{tt}f{half}")
            for cc in range(CCH):
                nc.tensor.matmul(
                    ps,
                    lhsT=mm(ao_all[(b, cc)][:, tt * 128:(tt + 1) * 128]),
                    rhs=mm(wproj_sb[cc][:, half * PH:(half + 1) * PH]),
                    start=(cc == 0), stop=(cc == CCH - 1))
            yt = yp.tile([128, PH], FP32, tag="y", name=f"y_b{b}t{tt}f{half}")
            with tc.high_priority(offset=300):
                nc.vector.tensor_add(yt, ps,
                                     bias_sb[:, half * PH:(half + 1) * PH])
            nc.sync.dma_start(
                out=out_d[b * n + tt * 128:b * n + (tt + 1) * 128,
                          half * PH:(half + 1) * PH],
                in_=yt)

        def emit_S(b, hp, kt):
            """S^T matmuls + exp for both heads of one 128-key tile."""
            qb = qt_all[(b, hp)]
            kb = kt_all[(b, hp)]
            for head in range(2):
                p0 = head * 64
                sps = ps_s.tile([128, n], FP32, tag="s",
                                name=f"s_b{b}hp{hp}k{kt}h{head}")
                for qn in range(NQ):
                    nc.tensor.matmul(
                        sps[:, qn * 512:(qn + 1) * 512],
                        lhsT=mm(kb[p0:p0 + 64, kt * 128:(kt + 1) * 128]),
                        rhs=mm(qb[p0:p0 + 64, qn * 512:(qn + 1) * 512]),
                        start=True, stop=True)
                et = ep.tile([128, n], SD, tag=f"e{head}",
                             name=f"e_b{b}hp{hp}k{kt}h{head}")
                nc.scalar.activation(et, sps, Exp, scale=scale)
                e_all[(b, hp, kt, head)] = et

        def emit_U_chunk(b, hp, kts):
            """U matmuls for key-tiles `kts`; their E tiles are all ready."""
            if (b, hp) not in u_ps:
                u_ps[(b, hp)] = [[ps_f.tile([VW, 512], FP32, tag="u",
                                            name=f"u_b{b}hp{hp}h{hh}q{qn}")
                                  for qn in range(NQ)] for hh in range(2)]
            ups = u_ps[(b, hp)]
            for kt in kts:
                for head in range(2):
                    hh = 2 * hp + head
                    et = e_all.pop((b, hp, kt, head))
                    for qn in range(NQ):
                        nc.tensor.matmul(
                            ups[head][qn],
                            lhsT=mm(v_all[b][kt][:, hh * VW:hh * VW + VW]),
                            rhs=mm(et[:, qn * 512:(qn + 1) * 512]),
                            start=(kt == 0), stop=(kt == NKT - 1))

        def emit_norm(b, hp, last=False):
            """Evacuate U psum, divide by Z, build aoT tile for proj."""
            ups = u_ps.pop((b, hp))
            ao = aop.tile([128, n], SD, tag=f"ao{hp}", name=f"ao_b{b}hp{hp}")
            ao_all[(b, hp)] = ao
            if last:
                # per-512-half chains so the warm proj groups (which read
                # only ao[:, 0:512]) can finish as soon as possible
                usb_l = {}
                for head in (1, 0):
                    usb_l[head] = smp.tile([VW, n], FP32, tag=f"usb{head}",
                                           name=f"usbL_b{b}hp{hp}h{head}")
                for qn in range(NQ):
                    sl = slice(qn * 512, (qn + 1) * 512)
                    for head in (1, 0):
                        with tc.high_priority(offset=300):
                            nc.vector.tensor_copy(usb_l[head][:, sl],
                                                  ups[head][qn])
                    for head in (1, 0):
                        usb = usb_l[head]
                        z1 = smp.tile([1, 512], FP32, tag=f"z1{head}",
                                      bufs=1, name=f"z1L_b{b}hp{hp}h{head}q{qn}")
                        nc.gpsimd.dma_start(out=z1, in_=usb[hd:hd + 1, sl])
                        rb = smp.tile([64, 512], FP32, tag=f"rb{head}",
                                      name=f"rbL_b{b}hp{hp}h{head}q{qn}")
                        nc.gpsimd.partition_broadcast(rb, z1)
                        nc.vector.reciprocal_approx_fast(rb, rb)
                        if head == 0:
                            nc.vector.tensor_mul(ao[0:64, sl],
                                                 usb[0:hd, sl], rb)
                        else:
                            sc = smp.tile([64, 512], SD, tag="sc",
                                          name=f"scL_b{b}hp{hp}q{qn}")
                            nc.vector.tensor_mul(sc, usb[0:hd, sl], rb)
                            nc.gpsimd.dma_start(out=ao[64:128, sl], in_=sc)
                return
            for head in (1, 0):
                usb = smp.tile([VW, n], FP32, tag=f"usb{head}",
                               name=f"usb_b{b}hp{hp}h{head}")
                for qn in range(NQ):
                    with tc.high_priority(offset=300):
                        nc.vector.tensor_copy(
                            usb[:, qn * 512:(qn + 1) * 512], ups[head][qn])
                # Z row -> partition 0 (DMA), broadcast to 64 partitions
                # (gpsimd), then reciprocal on the full-width tile (the
                # custom DVE op mis-executes on 1-partition slices at
                # base partition != 0).
                z1 = smp.tile([1, n], FP32, tag=f"z1{head}", bufs=1,
                              name=f"z1_b{b}hp{hp}h{head}")
                nc.sync.dma_start(out=z1, in_=usb[hd:hd + 1, :])
                rb = smp.tile([64, n], FP32, tag=f"rb{head}",
                              name=f"rb_b{b}hp{hp}h{head}")
                nc.gpsimd.partition_broadcast(rb, z1)
                nc.vector.reciprocal_approx_fast(rb, rb)
                if head == 0:
                    nc.vector.tensor_mul(ao[0:64, :], usb[0:hd, :], rb)
                else:
                    sc = smp.tile([64, n], SD, tag="sc",
                                  name=f"sc_b{b}hp{hp}")
                    nc.vector.tensor_mul(sc, usb[0:hd, :], rb)
                    nc.sync.dma_start(out=ao[64:128, :], in_=sc)

        # ---------------- filler schedule ----------------------------------
        # per-unit list of thunks run between S groups of that unit
        fillers = [[] for _ in range(NU)]

        def add_qk_fillers(i, b, hp):
            for dst in range(2):
                for qn in range(NQ):
                    fillers[i].append(
                        lambda b=b, hp=hp, dst=dst, qn=qn:
                        emit_qk_group(b, hp, dst, qn))

        # unit 0: v(b0) fully + qk(unit1)
        for tt in range(NTT):
            for half in range(2):
                fillers[0].append(
                    lambda tt=tt, half=half: emit_v_group(0, tt, half))
        add_qk_fillers(0, *units[1])
        # units 1..4: qk(next) + v(b1) spread 4 per unit
        for i in range(1, 5):
            add_qk_fillers(i, *units[i + 1])
        vq = [(tt, half) for tt in range(NTT) for half in range(2)]
        for j, (tt, half) in enumerate(vq):
            fillers[1 + j // 4].append(
                lambda tt=tt, half=half: emit_v_group(1, tt, half))
        # units 5..10: qk(next)
        for i in range(5, 11):
            add_qk_fillers(i, *units[i + 1])
        # units 7..10: proj(b0)  (all ao(b0) ready after norm(u5) in unit 6;
        # unit 11 keeps its flex psum free for in-unit U accumulation)
        pq = [(tt, half) for tt in range(NTT) for half in range(2)]
        for j, (tt, half) in enumerate(pq):
            fillers[7 + j % 4].append(
                lambda tt=tt, half=half: emit_proj_group(0, tt, half))

        # ---------------- main schedule ------------------------------------
        # prologue: qk(unit0)
        b0, hp0 = units[0]
        for dst in range(2):
            for qn in range(NQ):
                emit_qk_group(b0, hp0, dst, qn)

        for i, (b, hp) in enumerate(units):
            fl = list(fillers[i])
            prev = units[i - 1] if i > 0 else None
            # S(k0) first so the Scalar engine stays fed across the boundary;
            # previous unit's U matmuls run in chunks between S groups so
            # exp never starves and every U operand is long since ready.
            emit_S(b, hp, 0)
            start_kt = 1
            if prev is not None:
                for j, (k0, k1) in enumerate(((0, 2), (2, 4), (4, 6), (6, 8))):
                    emit_U_chunk(prev[0], prev[1], range(k0, k1))
                    if j < 3:
                        emit_S(b, hp, j + 1)
                emit_norm(*prev)
                start_kt = 4
            # spread fillers across the remaining kt slots; the last unit
            # instead runs its own U matmuls in-unit at lag 4
            nslots = NKT - start_kt
            tot = len(fl)
            for kt in range(start_kt, NKT):
                emit_S(b, hp, kt)
                if i == NU - 1 and kt >= 4:
                    emit_U_chunk(b, hp, [kt - 4])
                j = kt - start_kt
                k = (tot * (j + 1)) // nslots - (tot * j) // nslots
                for _ in range(k):
                    if fl:
                        fl.pop(0)()

        # epilogue: last unit's remaining U, then proj(b1). Six proj
        # groups pre-accumulate their first 5 cc-chunks (2 on the freed
        # S slots, 4 on flex once the U evacuation frees them) while the
        # final norm chain (DVE/gpsimd) drains, so the PE never idles.
        pb, php = units[-1]
        emit_U_chunk(pb, php, range(4, NKT))
        emit_norm(pb, php, last=True)
        warm = [(0, 0), (0, 1), (1, 0), (1, 1), (2, 0), (2, 1)]
        warm_ps = {}
        for j, (tt, half) in enumerate(warm):
            pool, tag = (ps_s, "s") if j < 2 else (ps_f, "u")
            ps = pool.tile([128, PH], FP32, tag=tag,
                           name=f"ypsw_b1t{tt}f{half}")
            warm_ps[(tt, half)] = ps
            for cc in range(CCH - 1):
                nc.tensor.matmul(
                    ps,
                    lhsT=mm(ao_all[(1, cc)][:, tt * 128:(tt + 1) * 128]),
                    rhs=mm(wproj_sb[cc][:, half * PH:(half + 1) * PH]),
                    start=(cc == 0), stop=False)
        for tt, half in warm:
            ps = warm_ps[(tt, half)]
            cc = CCH - 1
            nc.tensor.matmul(
                ps,
                lhsT=mm(ao_all[(1, cc)][:, tt * 128:(tt + 1) * 128]),
                rhs=mm(wproj_sb[cc][:, half * PH:(half + 1) * PH]),
                start=False, stop=True)
            yt = yp.tile([128, PH], FP32, tag="y", name=f"yw_b1t{tt}f{half}")
            with tc.high_priority(offset=300):
                nc.vector.tensor_add(yt, ps,
                                     bias_sb[:, half * PH:(half + 1) * PH])
            nc.sync.dma_start(
                out=out_d[n + tt * 128:n + (tt + 1) * 128,
                          half * PH:(half + 1) * PH],
                in_=yt)
        for tt in range(NTT):
            for half in range(2):
                if (tt, half) not in warm_ps:
                    emit_proj_group(1, tt, half)

    nc.compile()
    return nc


_NC_CACHE = {}


def _get_nc(compute=COMPUTE):
    if compute not in _NC_CACHE:
        _NC_CACHE[compute] = build_attention_nc(compute)
    return _NC_CACHE[compute]


def make_in_maps(x, W_qkv, W_proj, b_proj, compute=None):
    compute = compute or COMPUTE
    if compute == "bf16":
        import ml_dtypes
        sd = ml_dtypes.bfloat16
    else:
        sd = np.float32
    x = np.asarray(x, dtype=np.float32)
    W_qkv = np.ascontiguousarray(np.asarray(W_qkv, dtype=np.float32)).astype(sd)
    W_proj = np.ascontiguousarray(np.asarray(W_proj, dtype=np.float32)).astype(sd)
    bias = np.ascontiguousarray(
        np.broadcast_to(np.asarray(b_proj, dtype=np.float32), (128, C)))
    in_maps = []
    for i in range(NCORES):
        shard = x[i * BL:(i + 1) * BL]                      # [BL, N, C]
        xT = np.ascontiguousarray(shard.transpose(2, 0, 1).reshape(C, T)).astype(sd)
        in_maps.append({"xT": xT, "w_qkv": W_qkv, "w_proj": W_proj,
                        "bias": bias})
    return in_maps


def kernel(x, W_qkv, W_proj, b_proj):
    from concourse.bass_utils import run_bass_kernel_spmd

    nc = _get_nc()
    in_maps = make_in_maps(x, W_qkv, W_proj, b_proj)
    res = run_bass_kernel_spmd(nc, in_maps, core_ids=list(range(NCORES)))
    outs = [res.results[i]["out"].reshape(BL, N, C) for i in range(NCORES)]
    return np.concatenate(outs, axis=0).astype(np.float32)


if __name__ == "__main__":
    nc = build_attention_nc()
    print("built ok")
